# revision 34
# baseline (speedup 1.0000x reference)
"""AnyPrecisionLinear (4-bit LUT dequant + matmul) on 8 TRN2 NeuronCores.

y = x @ W.T with W[o,i] = lut[o, qweight[o,i]].

Sharding: column-parallel over out_features (1376 rows/core, padded to 1408).
Per core pipeline (per 128-row o-tile):
  - dequant: 8 custom fused DVE pair-ops: acc' = acc + (q==k)*lut_k + (q==k+1)*lut_{k+1}
    emitted in 4 column-chunks of 1024 so downstream consumers can start early
  - transpose W chunks into [i, o] layout (DMA x-bar transpose, or PE identity
    matmul + ACT copy)
  - matmul: W.T stationary [128i x 128o], moving x.T [128i x 512b], PSUM fp32
  - y.T written fp16; host concatenates, slices padding, transposes back.

"flow" schedule: dequant instruction groups are emitted eagerly (DVE runs
back-to-back from t~=1.5us), transposes ride the DMA engines, and the PE
stream is pure matmuls whose chunk-level semaphores let the first o-tile's
matmuls start ~10us in (vs ~48us when the whole dequant chain is serial).
o-tiles join the rotating batch-block rounds as their dequant completes
(estimated with a static timing model; estimates only affect instruction
order, never correctness - Tile semaphores enforce the real dependencies).
"""

import re
import sys

sys.path.insert(0, "/opt/trn_rl_repo")

import numpy as np

import concourse.mybir as mybir
import concourse.tile as tile
from concourse import bacc
from concourse.masks import make_identity

FP16 = mybir.dt.float16
FP32 = mybir.dt.float32

NCORES = 8
IN = 4096
BATCH = 4096
OUT = 11008
OUT_SLICE = OUT // NCORES  # 1376
OTILES = 11
OUT_PAD = OTILES * 128  # 1408
NK = 16
BC = 512
NBC = BATCH // BC  # 8
ITILES = IN // 128  # 32
ALU = mybir.AluOpType

OPT = {
    "sched": "flow",  # "flow" | "wavefront"
    "transpose": "pe2",  # "pe2" (packed-PSUM PE) | "dma"
    "x_splits": 8,
    "xbufs": 3,
    "chunks": 4,
    "join_la": 1.5,
    "accf_bufs": 8,
    "qbufs": 6,
    "q_splits": 4,
    "bridge": (3.0, 2.5, 12),
}


def _register_pair_op():
    from concourse.dve_ops import (
        OPS,
        _SUB_OPCODE_FOR_NAME,
        _CUSTOM_DVE_ROW_BASE,
        CUSTOM_DVE_SPECS,
        DveOp,
    )
    from concourse.dve_spec import Spec, Src0, Src1, C0, C1, C2, One, eq

    name = "ANYPREC_PAIR_ANT"
    if name in _SUB_OPCODE_FOR_NAME:
        return next(op for op in OPS if op.name == name)

    body = (Src0 + eq(Src1, C2) * C0) + eq(Src1, C2 + One) * C1

    def _ref(in0, in1, s0, s1, imm2):
        dd = in1.astype(np.float32) - imm2
        return (
            in0.astype(np.float32)
            + (dd == 0.0) * np.asarray(s0, np.float32)
            + (dd == 1.0) * np.asarray(s1, np.float32)
        ).astype(np.float32)

    op = DveOp(name, Spec(body=body, reference=_ref), subdim=False, uops_sha={})
    _SUB_OPCODE_FOR_NAME[name] = _CUSTOM_DVE_ROW_BASE + len(OPS)
    OPS.append(op)
    CUSTOM_DVE_SPECS[name] = op.spec
    for ver in ("v3",):
        try:
            op.compile(ver)
        except ValueError as e:
            m = re.search(r"\(%s: ([0-9a-f]+) " % ver, str(e))
            if not m:
                raise
            op.uops_sha[ver] = m.group(1)
            op.compile(ver)
    return op


def _build(opt=None):
    opt = {**OPT, **(opt or {})}
    pair_op = _register_pair_op()
    nc = bacc.Bacc(None, target_bir_lowering=False, debug=False)
    xt_ext = nc.declare_dram_parameter("xt", [IN, BATCH], FP16, isOutput=False)
    qf_ext = nc.declare_dram_parameter("qf", [OUT_PAD, IN], FP16, isOutput=False)
    # lut pre-packed on host to partition-major [128, OTILES*NK]
    lut_ext = nc.declare_dram_parameter("lut", [128, OTILES * NK], FP32, isOutput=False)
    yt_ext = nc.declare_dram_parameter("yt", [OUT_PAD, BATCH], FP16, isOutput=True)

    CHN = opt["chunks"]
    CW = IN // CHN  # chunk width (columns of q / acc)
    ITC = CW // 128  # itiles per chunk
    tp_mode = opt["transpose"]

    with tile.TileContext(nc) as tc:
        with (
            tc.tile_pool(name="const", bufs=1) as const_pool,
            tc.tile_pool(name="qp", bufs=opt["qbufs"]) as q_pool,
            tc.tile_pool(name="lutp", bufs=OTILES) as lut_pool,
            tc.tile_pool(name="accw", bufs=3) as accw_pool,
            tc.tile_pool(name="accf", bufs=opt["accf_bufs"]) as accf_pool,
            tc.tile_pool(name="wtp", bufs=8) as wt_pool,
            tc.tile_pool(name="xp", bufs=opt["xbufs"]) as x_pool,
            tc.tile_pool(name="ysp", bufs=4) as ys_pool,
            tc.tile_pool(name="tpp", bufs=3, space="PSUM") as tp_pool,
            tc.tile_pool(name="ypp", bufs=4, space="PSUM") as y_pool,
            tc.tile_pool(name="wpp", bufs=1, space="PSUM") as warm_pool,
        ):
            if tp_mode == "pe2":
                ident = const_pool.tile([128, 128], FP16)
                make_identity(nc, ident[:])
            zeros = const_pool.tile([128, CW], FP16)
            nc.vector.memset(zeros[:], 0.0)

            qtiles = {}
            state = {"qnext": 0}

            def q_prefetch(upto):
                while state["qnext"] < min(upto, OTILES * CHN):
                    g = state["qnext"]
                    og, c = g // CHN, g % CHN
                    qt = q_pool.tile([128, CW], FP16, tag="q")
                    nsp = opt["q_splits"]
                    w = CW // nsp
                    # first chunks via SWDGE (parallel queues, ~4x lower
                    # latency than serialized HWDGE) so DVE starts early
                    eng = nc.gpsimd if g < 2 else nc.sync
                    for s in range(nsp):
                        eng.dma_start(
                            out=qt[:, s * w : (s + 1) * w],
                            in_=qf_ext[
                                og * 128 : (og + 1) * 128,
                                c * CW + s * w : c * CW + (s + 1) * w,
                            ],
                        )
                    qtiles[(og, c)] = qt
                    state["qnext"] = g + 1

            q_prefetch(1)
            # all LUT rows in one contiguous DMA (host pre-packed): [128, 176]
            lut_all = lut_pool.tile([128, OTILES * NK], FP32, tag="lt")
            nc.sync.dma_start(out=lut_all[:], in_=lut_ext[:, :])
            q_prefetch(2)

            wts = {}
            accs = {}  # (og, seg_idx) -> (final acc tile, col0, width)
            tp_cnt = {}  # og -> number of segments transposed
            segs = {}  # og -> list of (col0, width)
            d_seg = {}  # (og, seg_idx) -> modeled DVE completion time (us)
            dve_clock = [1.5]

            def og_segments(og):
                # og0 in fine segments so its first matmuls (and hence PE
                # warm-up) start early; later o-tiles in full chunks
                if og == 0:
                    return [(0, 512), (512, 512)] + [
                        (c0, 1024) for c0 in range(1024, IN, 1024)
                    ]
                return [(c * CW, CW) for c in range(CHN)]

            def emit_deq_og(og):
                """Dequant (DVE only) for a whole o-tile, segment by segment."""
                wt = wt_pool.tile([128, IN], FP16, tag="wt")
                wts[og] = wt
                tp_cnt[og] = 0
                segs[og] = og_segments(og)
                for si, (c0, w) in enumerate(segs[og]):
                    dve_clock[0] += (w * 1.049e-3 + 0.27) * 8
                    d_seg[(og, si)] = dve_clock[0]
                    q_prefetch(og * CHN + c0 // CW + 4)
                    qc = qtiles[(og, c0 // CW)]
                    q = qc[:, c0 % CW : c0 % CW + w]
                    if c0 % CW + w == CW:
                        qtiles.pop((og, c0 // CW))
                    acc = None
                    for p in range(8):
                        pool = accf_pool if p == 7 else accw_pool
                        nacc = pool.tile([128, CW], FP16, tag="acc")
                        nc.vector._custom_dve(
                            pair_op,
                            out=nacc[:, :w],
                            in0=zeros[:, :w] if acc is None else acc[:, :w],
                            in1=q,
                            s0=lut_all[:, og * NK + 2 * p : og * NK + 2 * p + 1],
                            s1=lut_all[:, og * NK + 2 * p + 1 : og * NK + 2 * p + 2],
                            imm2=float(2 * p),
                        )
                        acc = nacc
                    if tp_mode == "dma":
                        nc.sync.dma_start_transpose(
                            out=wt[:, c0 : c0 + w].rearrange(
                                "p (t r) -> p t r", t=w // 128
                            ),
                            in_=acc[:, :w],
                        )
                        tp_cnt[og] = si + 1
                    else:
                        accs[(og, si)] = (acc, c0, w)

            def emit_tp_seg(og):
                """PE-transpose one pending segment into packed PSUM banks
                (<=1024 cols each), evacuate with wide ACT copies."""
                si = tp_cnt[og]
                if si >= len(segs[og]):
                    return
                acc, c0, w = accs.pop((og, si))
                for b0 in range(0, w, 1024):
                    bw = min(1024, w - b0)
                    tp = tp_pool.tile([128, bw], FP16, tag="tp")
                    for t in range(bw // 128):
                        nc.tensor.transpose(
                            tp[:, t * 128 : (t + 1) * 128],
                            acc[:, b0 + t * 128 : b0 + (t + 1) * 128],
                            ident[:],
                        )
                    nc.scalar.copy(
                        out=wts[og][:, c0 + b0 : c0 + b0 + bw], in_=tp[:, :bw]
                    )
                tp_cnt[og] = si + 1

            def emit_xblock(bc):
                xb = x_pool.tile([128, ITILES * BC], FP16, tag="xb")
                nsplit = opt["x_splits"]
                ichunk = ITILES // nsplit
                for s in range(nsplit):
                    nc.gpsimd.dma_start(
                        out=xb[:, s * ichunk * BC : (s + 1) * ichunk * BC].rearrange(
                            "p (i b) -> p i b", i=ichunk
                        ),
                        in_=xt_ext[
                            s * ichunk * 128 : (s + 1) * ichunk * 128,
                            bc * BC : (bc + 1) * BC,
                        ].rearrange("(i p) b -> p i b", p=128),
                    )
                return xb

            def emit_warm(n, ds):
                dp = warm_pool.tile([128, BC], FP32, tag="warm")
                for d in range(n):
                    nc.tensor.matmul(
                        dp[:],
                        lhsT=zeros[:, :128],
                        rhs=zeros[:, :ds],
                        start=(d == 0),
                        stop=(d == n - 1),
                    )

            def emit_mm(og, bc, xb, pe_now=1e9):
                yp = y_pool.tile([128, BC], FP32, tag="yp")
                sg = segs[og]
                starts = {c0 // 128: si for si, (c0, w) in enumerate(sg)}
                for i0 in range(ITILES):
                    si = starts.get(i0)
                    if si is not None and tp_mode != "dma":
                        # required segment, plus ahead-transposes only for
                        # segments whose dequant is predicted complete
                        t_here = pe_now + i0 * 0.216
                        if pe_now < 1e8 and opt["bridge"]:
                            thr, sub, cap = opt["bridge"]
                            stall = d_seg[(og, si)] - t_here
                            if stall > thr:
                                # bridge the predicted DVE wait with dummy
                                # matmuls so HAM keeps the PE at full clock
                                emit_warm(min(int((stall - sub) / 0.216), cap), BC)
                        while tp_cnt[og] < len(sg) and (
                            tp_cnt[og] <= si
                            or d_seg[(og, tp_cnt[og])] <= t_here - 1.0
                        ):
                            emit_tp_seg(og)
                    nc.tensor.matmul(
                        yp[:],
                        lhsT=wts[og][:, i0 * 128 : (i0 + 1) * 128],
                        rhs=xb[:, i0 * BC : (i0 + 1) * BC],
                        start=(i0 == 0),
                        stop=(i0 == ITILES - 1),
                    )
                ys = ys_pool.tile([128, BC], FP16, tag="ys")
                nc.scalar.copy(out=ys[:], in_=yp[:])
                nc.scalar.dma_start(
                    out=yt_ext[og * 128 : (og + 1) * 128, bc * BC : (bc + 1) * BC],
                    in_=ys[:],
                )

            if opt["sched"] == "flow":
                # ---- static timing model (us) -- drives emission order only
                UNIT_US = 32 * 0.216 + 0.05
                XB_US = 13.0

                def d_full(og):
                    return d_seg[(og, len(segs[og]) - 1)]

                q_prefetch(4)
                bc_done = [0] * OTILES
                units_emitted = [0] * OTILES
                joined = []
                next_join = 0
                deq_og = 0
                pe_t = 0.0
                xb_t = 0.0
                round_ends = []
                r = 0
                while min(bc_done) < 8:
                    # emit dequant for ogs whose wt buffer (8-deep pool) is free
                    while deq_og < OTILES and (
                        deq_og < 8 or units_emitted[deq_og - 8] == 8
                    ):
                        emit_deq_og(deq_og)
                        deq_og += 1
                    members = [og for og in joined if bc_done[og] < 8]
                    est = pe_t + len(members) * UNIT_US
                    while next_join < deq_og:
                        # starved rounds absorb a joiner's dequant stalls for
                        # free; busy rounds should not head-of-line block
                        la = 6.0 if len(members) <= 1 else opt["join_la"]
                        if members and d_full(next_join) > est + la:
                            break
                        joined.append(next_join)
                        members.append(next_join)
                        est += UNIT_US
                        next_join += 1
                    assert members, "flow schedule stalled"
                    bc = r % NBC
                    avail = round_ends[r - opt["xbufs"]] if r >= opt["xbufs"] else 0.0
                    xb_t = max(xb_t, avail) + XB_US
                    xb = emit_xblock(bc)
                    for i, og in enumerate(members):
                        start = pe_t
                        if i == 0:
                            start = max(start, xb_t)
                        emit_mm(og, bc, xb, pe_now=start)
                        end = start + UNIT_US
                        if bc_done[og] == 0:
                            end = max(end, d_full(og) + 2.0)
                        pe_t = end
                        bc_done[og] += 1
                        units_emitted[og] += 1
                    if len(members) == 1 and min(bc_done) < 8:
                        # xb-paced solo rounds idle ~6us -> HAM re-throttles;
                        # burn ~3us of dummy matmuls to keep the PE warm
                        emit_warm(14, BC)
                    round_ends.append(pe_t)
                    r += 1
            else:
                # original lockstep wavefront (whole-og dequant groups)
                s_of = list(range(OTILES))
                total_rounds = OTILES - 1 + NBC  # 18
                q_prefetch(4)
                emit_deq_og(0)
                for r in range(total_rounds):
                    if r + 1 < OTILES:
                        emit_deq_og(r + 1)
                    bc = r % NBC
                    units = [og for og in range(OTILES) if s_of[og] <= r < s_of[og] + NBC]
                    if not units:
                        continue
                    xb = emit_xblock(bc)
                    for og in units:
                        emit_mm(og, bc, xb)
    nc.finalize()
    return nc


_STATE = {}


def _get_compiled(opt=None):
    if "cb" in _STATE:
        return _STATE["cb"]
    import jax
    from jax.sharding import Mesh, PartitionSpec, NamedSharding
    from jax.experimental.shard_map import shard_map
    from concourse.bass2jax import (
        _bass_exec_p,
        install_neuronx_cc_hook,
        partition_id_tensor,
    )

    try:
        jax.config.update("jax_compilation_cache_dir", "/tmp/.anyprec_jaxcache")
        jax.config.update("jax_persistent_cache_min_compile_time_secs", 10)
        jax.config.update("jax_persistent_cache_min_entry_size_bytes", 0)
    except Exception:
        pass

    install_neuronx_cc_hook()
    nc = _build(opt)

    partition_name = nc.partition_id_tensor.name if nc.partition_id_tensor else None
    in_names, out_names, out_avals = [], [], []
    for alloc in nc.m.functions[0].allocations:
        if not isinstance(alloc, mybir.MemoryLocationSet):
            continue
        name = alloc.memorylocations[0].name
        if alloc.kind == "ExternalInput":
            if name != partition_name:
                in_names.append(name)
        elif alloc.kind == "ExternalOutput":
            out_names.append(name)
            out_avals.append(
                jax.core.ShapedArray(tuple(alloc.tensor_shape), mybir.dt.np(alloc.dtype))
            )
    all_in_names = in_names + out_names
    if partition_name is not None:
        all_in_names.append(partition_name)

    def _body(*args):
        operands = list(args)
        if partition_name is not None:
            operands.append(partition_id_tensor())
        return tuple(
            _bass_exec_p.bind(
                *operands,
                out_avals=tuple(out_avals),
                in_names=tuple(all_in_names),
                out_names=tuple(out_names),
                lowering_input_output_aliases=(),
                sim_require_finite=True,
                sim_require_nnan=True,
                nc=nc,
            )
        )

    devices = jax.devices()[:NCORES]
    mesh = Mesh(np.asarray(devices), ("core",))
    nin = len(in_names) + len(out_names)
    fn = jax.jit(
        shard_map(
            _body,
            mesh=mesh,
            in_specs=(PartitionSpec("core"),) * nin,
            out_specs=(PartitionSpec("core"),) * len(out_names),
            check_rep=False,
        ),
        keep_unused=True,
    )
    cb = {
        "fn": fn,
        "in_names": in_names,
        "out_names": out_names,
        "out_avals": out_avals,
        "sharding": NamedSharding(mesh, PartitionSpec("core")),
        "jax": jax,
    }
    _STATE["cb"] = cb
    return cb


def prepare_inputs(x, lut, qweight):
    x = np.asarray(x)
    lut = np.asarray(lut)
    qweight = np.asarray(qweight)
    xt = np.ascontiguousarray(x.astype(np.float16).T)  # [IN, BATCH]
    qf_full = qweight.astype(np.float16)  # exact for 0..15
    lut_full = lut.astype(np.float32)

    xt_cat = np.concatenate([xt] * NCORES, axis=0)
    qf_cat = np.zeros((NCORES * OUT_PAD, IN), np.float16)
    lut_cat = np.zeros((NCORES * 128, OTILES * NK), np.float32)
    for c in range(NCORES):
        r0, r1 = c * OUT_SLICE, (c + 1) * OUT_SLICE
        qf_cat[c * OUT_PAD : c * OUT_PAD + OUT_SLICE] = qf_full[r0:r1]
        # partition-major lut pack: lut_cat[c*128+p, og*NK+k] = lut[og*128+p, k]
        lp = np.zeros((OUT_PAD, NK), np.float32)
        lp[:OUT_SLICE] = lut_full[r0:r1]
        lut_cat[c * 128 : (c + 1) * 128] = (
            lp.reshape(OTILES, 128, NK).transpose(1, 0, 2).reshape(128, OTILES * NK)
        )
    return {"xt": xt_cat, "qf": qf_cat, "lut": lut_cat}


def run_device(arrs, bench_reps=0, opt=None):
    cb = _get_compiled(opt)
    jax = cb["jax"]
    dev_args = [jax.device_put(arrs[n], cb["sharding"]) for n in cb["in_names"]] + [
        jax.device_put(
            np.zeros((NCORES * a.shape[0], *a.shape[1:]), a.dtype), cb["sharding"]
        )
        for a in cb["out_avals"]
    ]
    jax.block_until_ready(dev_args)
    outs = cb["fn"](*dev_args)
    jax.block_until_ready(outs)
    result = np.asarray(outs[0])  # [8*OUT_PAD, BATCH] fp16

    timing = None
    if bench_reps:
        import time

        def run_n(n):
            best = None
            for _ in range(2):
                t0 = time.perf_counter()
                o = None
                for _ in range(n):
                    o = cb["fn"](*dev_args)
                jax.block_until_ready(o)
                dt = time.perf_counter() - t0
                best = dt if best is None else min(best, dt)
            return best

        n1, n2 = 10, 10 + bench_reps
        t1, t2 = run_n(n1), run_n(n2)
        timing = (t2 - t1) / (n2 - n1)
    return result, timing


def kernel(x, lut, qweight, w_bits=4, _bench_reps=0, _opt=None):
    arrs = prepare_inputs(x, lut, qweight)
    yt_cat, timing = run_device(arrs, bench_reps=_bench_reps, opt=_opt)
    yt = yt_cat.reshape(NCORES, OUT_PAD, BATCH)[:, :OUT_SLICE, :].reshape(OUT, BATCH)
    y = np.ascontiguousarray(yt.T)  # [BATCH, OUT] fp16
    if _bench_reps:
        kernel._last_timing = timing
    return y


# revision 36
# speedup vs baseline: 1.2620x; 1.2620x over previous
"""AnyPrecisionLinear (4-bit LUT dequant + matmul) on 8 TRN2 NeuronCores.

y = x @ W.T with W[o,i] = lut[o, qweight[o,i]].

Sharding: column-parallel over out_features (1376 rows/core, padded to 1408).
Per core pipeline (per 128-row o-tile):
  - dequant: 8 custom fused DVE pair-ops: acc' = acc + (q==k)*lut_k + (q==k+1)*lut_{k+1}
    emitted in 4 column-chunks of 1024 so downstream consumers can start early
  - transpose W chunks into [i, o] layout (DMA x-bar transpose, or PE identity
    matmul + ACT copy)
  - matmul: W.T stationary [128i x 128o], moving x.T [128i x 512b], PSUM fp32
  - y.T written fp16; host concatenates, slices padding, transposes back.

"flow" schedule: dequant instruction groups are emitted eagerly (DVE runs
back-to-back from t~=1.5us), transposes ride the DMA engines, and the PE
stream is pure matmuls whose chunk-level semaphores let the first o-tile's
matmuls start ~10us in (vs ~48us when the whole dequant chain is serial).
o-tiles join the rotating batch-block rounds as their dequant completes
(estimated with a static timing model; estimates only affect instruction
order, never correctness - Tile semaphores enforce the real dependencies).
"""

import re
import sys

sys.path.insert(0, "/opt/trn_rl_repo")

import numpy as np

import concourse.mybir as mybir
import concourse.tile as tile
from concourse import bacc
from concourse.masks import make_identity

FP16 = mybir.dt.float16
FP32 = mybir.dt.float32

NCORES = 8
IN = 4096
BATCH = 4096
OUT = 11008
OUT_SLICE = OUT // NCORES  # 1376
OTILES = 11
OUT_PAD = OTILES * 128  # 1408
NK = 16
BC = 512
NBC = BATCH // BC  # 8
ITILES = IN // 128  # 32
ALU = mybir.AluOpType

OPT = {
    "sched": "flow",  # "flow" | "wavefront"
    "transpose": "pe2",  # "pe2" (packed-PSUM PE) | "dma"
    "x_splits": 8,
    "xbufs": 3,
    "chunks": 4,
    "join_la": 1.5,
    "accf_bufs": 8,
    "qbufs": 6,
    "q_splits": 4,
    "bridge": (3.0, 2.5, 12),
    "tp_drain": True,
}


def _register_pair_op():
    from concourse.dve_ops import (
        OPS,
        _SUB_OPCODE_FOR_NAME,
        _CUSTOM_DVE_ROW_BASE,
        CUSTOM_DVE_SPECS,
        DveOp,
    )
    from concourse.dve_spec import Spec, Src0, Src1, C0, C1, C2, One, eq

    name = "ANYPREC_PAIR_ANT"
    if name in _SUB_OPCODE_FOR_NAME:
        return next(op for op in OPS if op.name == name)

    body = (Src0 + eq(Src1, C2) * C0) + eq(Src1, C2 + One) * C1

    def _ref(in0, in1, s0, s1, imm2):
        dd = in1.astype(np.float32) - imm2
        return (
            in0.astype(np.float32)
            + (dd == 0.0) * np.asarray(s0, np.float32)
            + (dd == 1.0) * np.asarray(s1, np.float32)
        ).astype(np.float32)

    op = DveOp(name, Spec(body=body, reference=_ref), subdim=False, uops_sha={})
    _SUB_OPCODE_FOR_NAME[name] = _CUSTOM_DVE_ROW_BASE + len(OPS)
    OPS.append(op)
    CUSTOM_DVE_SPECS[name] = op.spec
    for ver in ("v3",):
        try:
            op.compile(ver)
        except ValueError as e:
            m = re.search(r"\(%s: ([0-9a-f]+) " % ver, str(e))
            if not m:
                raise
            op.uops_sha[ver] = m.group(1)
            op.compile(ver)
    return op


def _build(opt=None):
    opt = {**OPT, **(opt or {})}
    pair_op = _register_pair_op()
    nc = bacc.Bacc(None, target_bir_lowering=False, debug=False)
    xt_ext = nc.declare_dram_parameter("xt", [IN, BATCH], FP16, isOutput=False)
    qf_ext = nc.declare_dram_parameter("qf", [OUT_PAD, IN], FP16, isOutput=False)
    # lut pre-packed on host to partition-major [128, OTILES*NK]
    lut_ext = nc.declare_dram_parameter("lut", [128, OTILES * NK], FP32, isOutput=False)
    yt_ext = nc.declare_dram_parameter("yt", [OUT_PAD, BATCH], FP16, isOutput=True)

    CHN = opt["chunks"]
    CW = IN // CHN  # chunk width (columns of q / acc)
    ITC = CW // 128  # itiles per chunk
    tp_mode = opt["transpose"]

    with tile.TileContext(nc) as tc:
        with (
            tc.tile_pool(name="const", bufs=1) as const_pool,
            tc.tile_pool(name="qp", bufs=opt["qbufs"]) as q_pool,
            tc.tile_pool(name="lutp", bufs=OTILES) as lut_pool,
            tc.tile_pool(name="accw", bufs=3) as accw_pool,
            tc.tile_pool(name="accf", bufs=opt["accf_bufs"]) as accf_pool,
            tc.tile_pool(name="wtp", bufs=8) as wt_pool,
            tc.tile_pool(name="xp", bufs=opt["xbufs"]) as x_pool,
            tc.tile_pool(name="ysp", bufs=4) as ys_pool,
            tc.tile_pool(name="tpp", bufs=3, space="PSUM") as tp_pool,
            tc.tile_pool(name="ypp", bufs=4, space="PSUM") as y_pool,
            tc.tile_pool(name="wpp", bufs=1, space="PSUM") as warm_pool,
        ):
            if tp_mode == "pe2":
                ident = const_pool.tile([128, 128], FP16)
                make_identity(nc, ident[:])
            zeros = const_pool.tile([128, CW], FP16)
            nc.vector.memset(zeros[:], 0.0)

            qtiles = {}
            state = {"qnext": 0}

            def q_prefetch(upto):
                while state["qnext"] < min(upto, OTILES * CHN):
                    g = state["qnext"]
                    og, c = g // CHN, g % CHN
                    qt = q_pool.tile([128, CW], FP16, tag="q")
                    nsp = opt["q_splits"]
                    w = CW // nsp
                    # first chunks via SWDGE (parallel queues, ~4x lower
                    # latency than serialized HWDGE) so DVE starts early
                    eng = nc.gpsimd if g < 2 else nc.sync
                    for s in range(nsp):
                        eng.dma_start(
                            out=qt[:, s * w : (s + 1) * w],
                            in_=qf_ext[
                                og * 128 : (og + 1) * 128,
                                c * CW + s * w : c * CW + (s + 1) * w,
                            ],
                        )
                    qtiles[(og, c)] = qt
                    state["qnext"] = g + 1

            q_prefetch(1)
            # all LUT rows in one contiguous DMA (host pre-packed): [128, 176]
            lut_all = lut_pool.tile([128, OTILES * NK], FP32, tag="lt")
            nc.sync.dma_start(out=lut_all[:], in_=lut_ext[:, :])
            q_prefetch(2)

            wts = {}
            accs = {}  # (og, seg_idx) -> (final acc tile, col0, width)
            tp_cnt = {}  # og -> number of segments transposed
            segs = {}  # og -> list of (col0, width)
            d_seg = {}  # (og, seg_idx) -> modeled DVE completion time (us)
            dve_clock = [1.5]

            def og_segments(og):
                # og0 in fine segments so its first matmuls (and hence PE
                # warm-up) start early; later o-tiles in full chunks
                if og == 0:
                    return [(0, 512), (512, 512)] + [
                        (c0, 1024) for c0 in range(1024, IN, 1024)
                    ]
                return [(c * CW, CW) for c in range(CHN)]

            def emit_deq_og(og):
                """Dequant (DVE only) for a whole o-tile, segment by segment."""
                wt = wt_pool.tile([128, IN], FP16, tag="wt")
                wts[og] = wt
                tp_cnt[og] = 0
                segs[og] = og_segments(og)
                for si, (c0, w) in enumerate(segs[og]):
                    dve_clock[0] += (w * 1.049e-3 + 0.27) * 8
                    d_seg[(og, si)] = dve_clock[0]
                    q_prefetch(og * CHN + c0 // CW + 4)
                    qc = qtiles[(og, c0 // CW)]
                    q = qc[:, c0 % CW : c0 % CW + w]
                    if c0 % CW + w == CW:
                        qtiles.pop((og, c0 // CW))
                    acc = None
                    for p in range(8):
                        pool = accf_pool if p == 7 else accw_pool
                        nacc = pool.tile([128, CW], FP16, tag="acc")
                        nc.vector._custom_dve(
                            pair_op,
                            out=nacc[:, :w],
                            in0=zeros[:, :w] if acc is None else acc[:, :w],
                            in1=q,
                            s0=lut_all[:, og * NK + 2 * p : og * NK + 2 * p + 1],
                            s1=lut_all[:, og * NK + 2 * p + 1 : og * NK + 2 * p + 2],
                            imm2=float(2 * p),
                        )
                        acc = nacc
                    if tp_mode == "dma":
                        nc.sync.dma_start_transpose(
                            out=wt[:, c0 : c0 + w].rearrange(
                                "p (t r) -> p t r", t=w // 128
                            ),
                            in_=acc[:, :w],
                        )
                        tp_cnt[og] = si + 1
                    else:
                        accs[(og, si)] = (acc, c0, w)

            def emit_tp_seg(og):
                """PE-transpose one pending segment into packed PSUM banks
                (<=1024 cols each), evacuate with wide ACT copies."""
                si = tp_cnt[og]
                if si >= len(segs[og]):
                    return
                acc, c0, w = accs.pop((og, si))
                for b0 in range(0, w, 1024):
                    bw = min(1024, w - b0)
                    tp = tp_pool.tile([128, bw], FP16, tag="tp")
                    for t in range(bw // 128):
                        nc.tensor.transpose(
                            tp[:, t * 128 : (t + 1) * 128],
                            acc[:, b0 + t * 128 : b0 + (t + 1) * 128],
                            ident[:],
                        )
                    nc.scalar.copy(
                        out=wts[og][:, c0 + b0 : c0 + b0 + bw], in_=tp[:, :bw]
                    )
                tp_cnt[og] = si + 1

            def emit_xblock(bc):
                xb = x_pool.tile([128, ITILES * BC], FP16, tag="xb")
                nsplit = opt["x_splits"]
                ichunk = ITILES // nsplit
                for s in range(nsplit):
                    nc.gpsimd.dma_start(
                        out=xb[:, s * ichunk * BC : (s + 1) * ichunk * BC].rearrange(
                            "p (i b) -> p i b", i=ichunk
                        ),
                        in_=xt_ext[
                            s * ichunk * 128 : (s + 1) * ichunk * 128,
                            bc * BC : (bc + 1) * BC,
                        ].rearrange("(i p) b -> p i b", p=128),
                    )
                return xb

            def emit_warm(n, ds):
                dp = warm_pool.tile([128, BC], FP32, tag="warm")
                for d in range(n):
                    nc.tensor.matmul(
                        dp[:],
                        lhsT=zeros[:, :128],
                        rhs=zeros[:, :ds],
                        start=(d == 0),
                        stop=(d == n - 1),
                    )

            def emit_mm(og, bc, xb, pe_now=1e9):
                yp = y_pool.tile([128, BC], FP32, tag="yp")
                sg = segs[og]
                starts = {c0 // 128: si for si, (c0, w) in enumerate(sg)}
                for i0 in range(ITILES):
                    si = starts.get(i0)
                    if si is not None and tp_mode != "dma":
                        # required segment, plus ahead-transposes only for
                        # segments whose dequant is predicted complete
                        t_here = pe_now + i0 * 0.216
                        if pe_now < 1e8 and opt["bridge"]:
                            thr, sub, cap = opt["bridge"]
                            stall = d_seg[(og, si)] - t_here
                            if stall > thr:
                                # bridge the predicted DVE wait with dummy
                                # matmuls so HAM keeps the PE at full clock
                                emit_warm(min(int((stall - sub) / 0.216), cap), BC)
                        while tp_cnt[og] < len(sg) and (
                            tp_cnt[og] <= si
                            or d_seg[(og, tp_cnt[og])] <= t_here - 1.0
                        ):
                            emit_tp_seg(og)
                    nc.tensor.matmul(
                        yp[:],
                        lhsT=wts[og][:, i0 * 128 : (i0 + 1) * 128],
                        rhs=xb[:, i0 * BC : (i0 + 1) * BC],
                        start=(i0 == 0),
                        stop=(i0 == ITILES - 1),
                    )
                ys = ys_pool.tile([128, BC], FP16, tag="ys")
                nc.scalar.copy(out=ys[:], in_=yp[:])
                nc.scalar.dma_start(
                    out=yt_ext[og * 128 : (og + 1) * 128, bc * BC : (bc + 1) * BC],
                    in_=ys[:],
                )

            if opt["sched"] == "flow":
                # ---- static timing model (us) -- drives emission order only
                UNIT_US = 32 * 0.216 + 0.05
                XB_US = 13.0

                def d_full(og):
                    return d_seg[(og, len(segs[og]) - 1)]

                q_prefetch(4)
                bc_done = [0] * OTILES
                units_emitted = [0] * OTILES
                joined = []
                next_join = 0
                deq_og = 0
                pe_t = 0.0
                xb_t = 0.0
                round_ends = []
                r = 0
                while min(bc_done) < 8:
                    # emit dequant for ogs whose wt buffer (8-deep pool) is free
                    while deq_og < OTILES and (
                        deq_og < 8 or units_emitted[deq_og - 8] == 8
                    ):
                        emit_deq_og(deq_og)
                        deq_og += 1
                    members = [og for og in joined if bc_done[og] < 8]
                    est = pe_t + len(members) * UNIT_US
                    while next_join < deq_og:
                        # starved rounds absorb a joiner's dequant stalls for
                        # free; busy rounds should not head-of-line block
                        la = 6.0 if len(members) <= 1 else opt["join_la"]
                        if members and d_full(next_join) > est + la:
                            break
                        joined.append(next_join)
                        members.append(next_join)
                        est += UNIT_US
                        next_join += 1
                    assert members, "flow schedule stalled"
                    bc = r % NBC
                    avail = round_ends[r - opt["xbufs"]] if r >= opt["xbufs"] else 0.0
                    xb_t = max(xb_t, avail) + XB_US
                    xb = emit_xblock(bc)
                    for i, og in enumerate(members):
                        start = pe_t
                        if i == 0:
                            start = max(start, xb_t)
                        emit_mm(og, bc, xb, pe_now=start)
                        end = start + UNIT_US
                        if bc_done[og] == 0:
                            end = max(end, d_full(og) + 2.0)
                        pe_t = end
                        bc_done[og] += 1
                        units_emitted[og] += 1
                    drained = 0.0
                    if opt["tp_drain"] and len(members) <= 2:
                        # absorb future o-tiles' transposes into the predicted
                        # idle before the next round's xb lands, instead of
                        # paying for them later inside a PE-backlogged unit
                        idle = max(0.0, (xb_t + XB_US) - pe_t)
                        t_cur = pe_t
                        for og in range(deq_og):
                            while (
                                tp_cnt[og] < len(segs[og])
                                and d_seg[(og, tp_cnt[og])] <= t_cur - 0.5
                                and drained + 0.9 <= idle
                            ):
                                emit_tp_seg(og)
                                drained += 0.9
                                t_cur += 0.9
                    if len(members) == 1 and min(bc_done) < 8 and drained < 2.0:
                        # xb-paced solo rounds idle ~6us -> HAM re-throttles;
                        # burn ~3us of dummy matmuls to keep the PE warm
                        emit_warm(14, BC)
                    round_ends.append(pe_t)
                    r += 1
            else:
                # original lockstep wavefront (whole-og dequant groups)
                s_of = list(range(OTILES))
                total_rounds = OTILES - 1 + NBC  # 18
                q_prefetch(4)
                emit_deq_og(0)
                for r in range(total_rounds):
                    if r + 1 < OTILES:
                        emit_deq_og(r + 1)
                    bc = r % NBC
                    units = [og for og in range(OTILES) if s_of[og] <= r < s_of[og] + NBC]
                    if not units:
                        continue
                    xb = emit_xblock(bc)
                    for og in units:
                        emit_mm(og, bc, xb)
    nc.finalize()
    return nc


_STATE = {}


def _get_compiled(opt=None):
    if "cb" in _STATE:
        return _STATE["cb"]
    import jax
    from jax.sharding import Mesh, PartitionSpec, NamedSharding
    from jax.experimental.shard_map import shard_map
    from concourse.bass2jax import (
        _bass_exec_p,
        install_neuronx_cc_hook,
        partition_id_tensor,
    )

    try:
        jax.config.update("jax_compilation_cache_dir", "/tmp/.anyprec_jaxcache")
        jax.config.update("jax_persistent_cache_min_compile_time_secs", 10)
        jax.config.update("jax_persistent_cache_min_entry_size_bytes", 0)
    except Exception:
        pass

    install_neuronx_cc_hook()
    nc = _build(opt)

    partition_name = nc.partition_id_tensor.name if nc.partition_id_tensor else None
    in_names, out_names, out_avals = [], [], []
    for alloc in nc.m.functions[0].allocations:
        if not isinstance(alloc, mybir.MemoryLocationSet):
            continue
        name = alloc.memorylocations[0].name
        if alloc.kind == "ExternalInput":
            if name != partition_name:
                in_names.append(name)
        elif alloc.kind == "ExternalOutput":
            out_names.append(name)
            out_avals.append(
                jax.core.ShapedArray(tuple(alloc.tensor_shape), mybir.dt.np(alloc.dtype))
            )
    all_in_names = in_names + out_names
    if partition_name is not None:
        all_in_names.append(partition_name)

    def _body(*args):
        operands = list(args)
        if partition_name is not None:
            operands.append(partition_id_tensor())
        return tuple(
            _bass_exec_p.bind(
                *operands,
                out_avals=tuple(out_avals),
                in_names=tuple(all_in_names),
                out_names=tuple(out_names),
                lowering_input_output_aliases=(),
                sim_require_finite=True,
                sim_require_nnan=True,
                nc=nc,
            )
        )

    devices = jax.devices()[:NCORES]
    mesh = Mesh(np.asarray(devices), ("core",))
    nin = len(in_names) + len(out_names)
    fn = jax.jit(
        shard_map(
            _body,
            mesh=mesh,
            in_specs=(PartitionSpec("core"),) * nin,
            out_specs=(PartitionSpec("core"),) * len(out_names),
            check_rep=False,
        ),
        keep_unused=True,
    )
    cb = {
        "fn": fn,
        "in_names": in_names,
        "out_names": out_names,
        "out_avals": out_avals,
        "sharding": NamedSharding(mesh, PartitionSpec("core")),
        "jax": jax,
    }
    _STATE["cb"] = cb
    return cb


def prepare_inputs(x, lut, qweight):
    x = np.asarray(x)
    lut = np.asarray(lut)
    qweight = np.asarray(qweight)
    xt = np.ascontiguousarray(x.astype(np.float16).T)  # [IN, BATCH]
    qf_full = qweight.astype(np.float16)  # exact for 0..15
    lut_full = lut.astype(np.float32)

    xt_cat = np.concatenate([xt] * NCORES, axis=0)
    qf_cat = np.zeros((NCORES * OUT_PAD, IN), np.float16)
    lut_cat = np.zeros((NCORES * 128, OTILES * NK), np.float32)
    for c in range(NCORES):
        r0, r1 = c * OUT_SLICE, (c + 1) * OUT_SLICE
        qf_cat[c * OUT_PAD : c * OUT_PAD + OUT_SLICE] = qf_full[r0:r1]
        # partition-major lut pack: lut_cat[c*128+p, og*NK+k] = lut[og*128+p, k]
        lp = np.zeros((OUT_PAD, NK), np.float32)
        lp[:OUT_SLICE] = lut_full[r0:r1]
        lut_cat[c * 128 : (c + 1) * 128] = (
            lp.reshape(OTILES, 128, NK).transpose(1, 0, 2).reshape(128, OTILES * NK)
        )
    return {"xt": xt_cat, "qf": qf_cat, "lut": lut_cat}


def run_device(arrs, bench_reps=0, opt=None):
    cb = _get_compiled(opt)
    jax = cb["jax"]
    dev_args = [jax.device_put(arrs[n], cb["sharding"]) for n in cb["in_names"]] + [
        jax.device_put(
            np.zeros((NCORES * a.shape[0], *a.shape[1:]), a.dtype), cb["sharding"]
        )
        for a in cb["out_avals"]
    ]
    jax.block_until_ready(dev_args)
    outs = cb["fn"](*dev_args)
    jax.block_until_ready(outs)
    result = np.asarray(outs[0])  # [8*OUT_PAD, BATCH] fp16

    timing = None
    if bench_reps:
        import time

        def run_n(n):
            best = None
            for _ in range(2):
                t0 = time.perf_counter()
                o = None
                for _ in range(n):
                    o = cb["fn"](*dev_args)
                jax.block_until_ready(o)
                dt = time.perf_counter() - t0
                best = dt if best is None else min(best, dt)
            return best

        n1, n2 = 10, 10 + bench_reps
        t1, t2 = run_n(n1), run_n(n2)
        timing = (t2 - t1) / (n2 - n1)
    return result, timing


def kernel(x, lut, qweight, w_bits=4, _bench_reps=0, _opt=None):
    arrs = prepare_inputs(x, lut, qweight)
    yt_cat, timing = run_device(arrs, bench_reps=_bench_reps, opt=_opt)
    yt = yt_cat.reshape(NCORES, OUT_PAD, BATCH)[:, :OUT_SLICE, :].reshape(OUT, BATCH)
    y = np.ascontiguousarray(yt.T)  # [BATCH, OUT] fp16
    if _bench_reps:
        kernel._last_timing = timing
    return y


# revision 41
# speedup vs baseline: 1.4659x; 1.1615x over previous
"""AnyPrecisionLinear (4-bit LUT dequant + matmul) on 8 TRN2 NeuronCores.

y = x @ W.T with W[o,i] = lut[o, qweight[o,i]].

Sharding: column-parallel over out_features (1376 rows/core, padded to 1408).
Per core pipeline (per 128-row o-tile):
  - dequant: 8 custom fused DVE pair-ops: acc' = acc + (q==k)*lut_k + (q==k+1)*lut_{k+1}
    emitted in 4 column-chunks of 1024 so downstream consumers can start early
  - transpose W chunks into [i, o] layout (DMA x-bar transpose, or PE identity
    matmul + ACT copy)
  - matmul: W.T stationary [128i x 128o], moving x.T [128i x 512b], PSUM fp32
  - y.T written fp16; host concatenates, slices padding, transposes back.

"flow" schedule: dequant instruction groups are emitted eagerly (DVE runs
back-to-back from t~=1.5us), transposes ride the DMA engines, and the PE
stream is pure matmuls whose chunk-level semaphores let the first o-tile's
matmuls start ~10us in (vs ~48us when the whole dequant chain is serial).
o-tiles join the rotating batch-block rounds as their dequant completes
(estimated with a static timing model; estimates only affect instruction
order, never correctness - Tile semaphores enforce the real dependencies).
"""

import re
import sys

sys.path.insert(0, "/opt/trn_rl_repo")

import numpy as np

import concourse.mybir as mybir
import concourse.tile as tile
from concourse import bacc
from concourse.masks import make_identity

FP16 = mybir.dt.float16
FP32 = mybir.dt.float32

NCORES = 8
IN = 4096
BATCH = 4096
OUT = 11008
OUT_SLICE = OUT // NCORES  # 1376
OTILES = 11
OUT_PAD = OTILES * 128  # 1408
NK = 16
BC = 512
NBC = BATCH // BC  # 8
ITILES = IN // 128  # 32
ALU = mybir.AluOpType

OPT = {
    "sched": "flow",  # "flow" | "wavefront"
    "transpose": "pe2",  # "pe2" (packed-PSUM PE) | "dma"
    "x_splits": 8,
    "xbufs": 3,
    "chunks": 4,
    "join_la": 1.5,
    "accf_bufs": 8,
    "qbufs": 6,
    "q_splits": 4,
    "bridge": (3.0, 2.5, 12),
    "tp_drain": True,
    "x_layout": "tiled",  # "tiled" (host pre-tiled, contiguous loads) | "gather"
}


def _register_pair_op():
    from concourse.dve_ops import (
        OPS,
        _SUB_OPCODE_FOR_NAME,
        _CUSTOM_DVE_ROW_BASE,
        CUSTOM_DVE_SPECS,
        DveOp,
    )
    from concourse.dve_spec import Spec, Src0, Src1, C0, C1, C2, One, eq

    name = "ANYPREC_PAIR_ANT"
    if name in _SUB_OPCODE_FOR_NAME:
        return next(op for op in OPS if op.name == name)

    body = (Src0 + eq(Src1, C2) * C0) + eq(Src1, C2 + One) * C1

    def _ref(in0, in1, s0, s1, imm2):
        dd = in1.astype(np.float32) - imm2
        return (
            in0.astype(np.float32)
            + (dd == 0.0) * np.asarray(s0, np.float32)
            + (dd == 1.0) * np.asarray(s1, np.float32)
        ).astype(np.float32)

    op = DveOp(name, Spec(body=body, reference=_ref), subdim=False, uops_sha={})
    _SUB_OPCODE_FOR_NAME[name] = _CUSTOM_DVE_ROW_BASE + len(OPS)
    OPS.append(op)
    CUSTOM_DVE_SPECS[name] = op.spec
    for ver in ("v3",):
        try:
            op.compile(ver)
        except ValueError as e:
            m = re.search(r"\(%s: ([0-9a-f]+) " % ver, str(e))
            if not m:
                raise
            op.uops_sha[ver] = m.group(1)
            op.compile(ver)
    return op


def _build(opt=None):
    opt = {**OPT, **(opt or {})}
    pair_op = _register_pair_op()
    nc = bacc.Bacc(None, target_bir_lowering=False, debug=False)
    if (opt or OPT).get("x_layout", OPT["x_layout"]) == "tiled":
        # host pre-tiled: xt[p, bc*ITILES*BC + i*BC + b] = x.T[i*128+p, bc*BC+b]
        xt_ext = nc.declare_dram_parameter(
            "xt", [128, NBC * ITILES * BC], FP16, isOutput=False
        )
    else:
        xt_ext = nc.declare_dram_parameter("xt", [IN, BATCH], FP16, isOutput=False)
    qf_ext = nc.declare_dram_parameter("qf", [OUT_PAD, IN], FP16, isOutput=False)
    # lut pre-packed on host to partition-major [128, OTILES*NK]
    lut_ext = nc.declare_dram_parameter("lut", [128, OTILES * NK], FP32, isOutput=False)
    yt_ext = nc.declare_dram_parameter("yt", [OUT_PAD, BATCH], FP16, isOutput=True)

    CHN = opt["chunks"]
    CW = IN // CHN  # chunk width (columns of q / acc)
    ITC = CW // 128  # itiles per chunk
    tp_mode = opt["transpose"]

    with tile.TileContext(nc) as tc:
        with (
            tc.tile_pool(name="const", bufs=1) as const_pool,
            tc.tile_pool(name="qp", bufs=opt["qbufs"]) as q_pool,
            tc.tile_pool(name="lutp", bufs=OTILES) as lut_pool,
            tc.tile_pool(name="accw", bufs=3) as accw_pool,
            tc.tile_pool(name="accf", bufs=opt["accf_bufs"]) as accf_pool,
            tc.tile_pool(name="wtp", bufs=8) as wt_pool,
            tc.tile_pool(name="xp", bufs=opt["xbufs"]) as x_pool,
            tc.tile_pool(name="ysp", bufs=4) as ys_pool,
            tc.tile_pool(name="tpp", bufs=3, space="PSUM") as tp_pool,
            tc.tile_pool(name="ypp", bufs=4, space="PSUM") as y_pool,
            tc.tile_pool(name="wpp", bufs=1, space="PSUM") as warm_pool,
        ):
            if tp_mode == "pe2":
                ident = const_pool.tile([128, 128], FP16)
                make_identity(nc, ident[:])
            zeros = const_pool.tile([128, CW], FP16)
            nc.vector.memset(zeros[:], 0.0)

            qtiles = {}
            state = {"qnext": 0}

            def q_prefetch(upto):
                while state["qnext"] < min(upto, OTILES * CHN):
                    g = state["qnext"]
                    og, c = g // CHN, g % CHN
                    qt = q_pool.tile([128, CW], FP16, tag="q")
                    nsp = opt["q_splits"]
                    w = CW // nsp
                    # first chunks via SWDGE (parallel queues, ~4x lower
                    # latency than serialized HWDGE) so DVE starts early
                    eng = nc.gpsimd if g < 2 else nc.sync
                    for s in range(nsp):
                        eng.dma_start(
                            out=qt[:, s * w : (s + 1) * w],
                            in_=qf_ext[
                                og * 128 : (og + 1) * 128,
                                c * CW + s * w : c * CW + (s + 1) * w,
                            ],
                        )
                    qtiles[(og, c)] = qt
                    state["qnext"] = g + 1

            q_prefetch(1)
            # all LUT rows in one contiguous DMA (host pre-packed): [128, 176]
            lut_all = lut_pool.tile([128, OTILES * NK], FP32, tag="lt")
            nc.sync.dma_start(out=lut_all[:], in_=lut_ext[:, :])
            q_prefetch(2)

            wts = {}
            accs = {}  # (og, seg_idx) -> (final acc tile, col0, width)
            tp_cnt = {}  # og -> number of segments transposed
            segs = {}  # og -> list of (col0, width)
            d_seg = {}  # (og, seg_idx) -> modeled DVE completion time (us)
            dve_clock = [1.5]

            def og_segments(og):
                # og0 in fine segments so its first matmuls (and hence PE
                # warm-up) start early; later o-tiles in full chunks
                if og == 0:
                    return [(0, 512), (512, 512)] + [
                        (c0, 1024) for c0 in range(1024, IN, 1024)
                    ]
                return [(c * CW, CW) for c in range(CHN)]

            def emit_deq_og(og):
                """Dequant (DVE only) for a whole o-tile, segment by segment."""
                wt = wt_pool.tile([128, IN], FP16, tag="wt")
                wts[og] = wt
                tp_cnt[og] = 0
                segs[og] = og_segments(og)
                for si, (c0, w) in enumerate(segs[og]):
                    dve_clock[0] += (w * 1.049e-3 + 0.27) * 8
                    d_seg[(og, si)] = dve_clock[0]
                    q_prefetch(og * CHN + c0 // CW + 4)
                    qc = qtiles[(og, c0 // CW)]
                    q = qc[:, c0 % CW : c0 % CW + w]
                    if c0 % CW + w == CW:
                        qtiles.pop((og, c0 // CW))
                    acc = None
                    for p in range(8):
                        pool = accf_pool if p == 7 else accw_pool
                        nacc = pool.tile([128, CW], FP16, tag="acc")
                        nc.vector._custom_dve(
                            pair_op,
                            out=nacc[:, :w],
                            in0=zeros[:, :w] if acc is None else acc[:, :w],
                            in1=q,
                            s0=lut_all[:, og * NK + 2 * p : og * NK + 2 * p + 1],
                            s1=lut_all[:, og * NK + 2 * p + 1 : og * NK + 2 * p + 2],
                            imm2=float(2 * p),
                        )
                        acc = nacc
                    if tp_mode == "dma":
                        nc.sync.dma_start_transpose(
                            out=wt[:, c0 : c0 + w].rearrange(
                                "p (t r) -> p t r", t=w // 128
                            ),
                            in_=acc[:, :w],
                        )
                        tp_cnt[og] = si + 1
                    else:
                        accs[(og, si)] = (acc, c0, w)

            def emit_tp_seg(og):
                """PE-transpose one pending segment into packed PSUM banks
                (<=1024 cols each), evacuate with wide ACT copies."""
                si = tp_cnt[og]
                if si >= len(segs[og]):
                    return
                acc, c0, w = accs.pop((og, si))
                for b0 in range(0, w, 1024):
                    bw = min(1024, w - b0)
                    tp = tp_pool.tile([128, bw], FP16, tag="tp")
                    for t in range(bw // 128):
                        nc.tensor.transpose(
                            tp[:, t * 128 : (t + 1) * 128],
                            acc[:, b0 + t * 128 : b0 + (t + 1) * 128],
                            ident[:],
                        )
                    nc.scalar.copy(
                        out=wts[og][:, c0 + b0 : c0 + b0 + bw], in_=tp[:, :bw]
                    )
                tp_cnt[og] = si + 1

            def emit_xblock(bc):
                xb = x_pool.tile([128, ITILES * BC], FP16, tag="xb")
                BLK = ITILES * BC
                if opt["x_layout"] == "tiled":
                    nsp = 4
                    sp = BLK // nsp
                    for s in range(nsp):
                        nc.gpsimd.dma_start(
                            out=xb[:, s * sp : (s + 1) * sp],
                            in_=xt_ext[:, bc * BLK + s * sp : bc * BLK + (s + 1) * sp],
                        )
                    return xb
                nsplit = opt["x_splits"]
                ichunk = ITILES // nsplit
                for s in range(nsplit):
                    nc.gpsimd.dma_start(
                        out=xb[:, s * ichunk * BC : (s + 1) * ichunk * BC].rearrange(
                            "p (i b) -> p i b", i=ichunk
                        ),
                        in_=xt_ext[
                            s * ichunk * 128 : (s + 1) * ichunk * 128,
                            bc * BC : (bc + 1) * BC,
                        ].rearrange("(i p) b -> p i b", p=128),
                    )
                return xb

            def emit_warm(n, ds):
                dp = warm_pool.tile([128, BC], FP32, tag="warm")
                for d in range(n):
                    nc.tensor.matmul(
                        dp[:],
                        lhsT=zeros[:, :128],
                        rhs=zeros[:, :ds],
                        start=(d == 0),
                        stop=(d == n - 1),
                    )

            def emit_mm(og, bc, xb, pe_now=1e9):
                yp = y_pool.tile([128, BC], FP32, tag="yp")
                sg = segs[og]
                starts = {c0 // 128: si for si, (c0, w) in enumerate(sg)}
                for i0 in range(ITILES):
                    si = starts.get(i0)
                    if si is not None and tp_mode != "dma":
                        # required segment, plus ahead-transposes only for
                        # segments whose dequant is predicted complete
                        t_here = pe_now + i0 * 0.216
                        if pe_now < 1e8 and opt["bridge"]:
                            thr, sub, cap = opt["bridge"]
                            stall = d_seg[(og, si)] - t_here
                            if stall > thr:
                                # bridge the predicted DVE wait with dummy
                                # matmuls so HAM keeps the PE at full clock
                                emit_warm(min(int((stall - sub) / 0.216), cap), BC)
                        while tp_cnt[og] < len(sg) and (
                            tp_cnt[og] <= si
                            or d_seg[(og, tp_cnt[og])] <= t_here - 1.0
                        ):
                            emit_tp_seg(og)
                    nc.tensor.matmul(
                        yp[:],
                        lhsT=wts[og][:, i0 * 128 : (i0 + 1) * 128],
                        rhs=xb[:, i0 * BC : (i0 + 1) * BC],
                        start=(i0 == 0),
                        stop=(i0 == ITILES - 1),
                    )
                ys = ys_pool.tile([128, BC], FP16, tag="ys")
                nc.scalar.copy(out=ys[:], in_=yp[:])
                nc.scalar.dma_start(
                    out=yt_ext[og * 128 : (og + 1) * 128, bc * BC : (bc + 1) * BC],
                    in_=ys[:],
                )

            if opt["sched"] == "flow":
                # ---- static timing model (us) -- drives emission order only
                UNIT_US = 32 * 0.216 + 0.05
                XB_US = 13.0

                def d_full(og):
                    return d_seg[(og, len(segs[og]) - 1)]

                q_prefetch(4)
                bc_done = [0] * OTILES
                units_emitted = [0] * OTILES
                joined = []
                next_join = 0
                deq_og = 0
                pe_t = 0.0
                xb_t = 0.0
                round_ends = []
                r = 0
                while min(bc_done) < 8:
                    # emit dequant for ogs whose wt buffer (8-deep pool) is free
                    while deq_og < OTILES and (
                        deq_og < 8 or units_emitted[deq_og - 8] == 8
                    ):
                        emit_deq_og(deq_og)
                        deq_og += 1
                    members = [og for og in joined if bc_done[og] < 8]
                    est = pe_t + len(members) * UNIT_US
                    while next_join < deq_og:
                        # starved rounds absorb a joiner's dequant stalls for
                        # free; busy rounds should not head-of-line block
                        la = 6.0 if len(members) <= 1 else opt["join_la"]
                        if members and d_full(next_join) > est + la:
                            break
                        joined.append(next_join)
                        members.append(next_join)
                        est += UNIT_US
                        next_join += 1
                    assert members, "flow schedule stalled"
                    bc = r % NBC
                    avail = round_ends[r - opt["xbufs"]] if r >= opt["xbufs"] else 0.0
                    xb_t = max(xb_t, avail) + XB_US
                    xb = emit_xblock(bc)
                    for i, og in enumerate(members):
                        start = pe_t
                        if i == 0:
                            start = max(start, xb_t)
                        emit_mm(og, bc, xb, pe_now=start)
                        end = start + UNIT_US
                        if bc_done[og] == 0:
                            end = max(end, d_full(og) + 2.0)
                        pe_t = end
                        bc_done[og] += 1
                        units_emitted[og] += 1
                    drained = 0.0
                    if opt["tp_drain"] and len(members) <= 2:
                        # absorb future o-tiles' transposes into the predicted
                        # idle before the next round's xb lands, instead of
                        # paying for them later inside a PE-backlogged unit
                        idle = max(0.0, (xb_t + XB_US) - pe_t)
                        t_cur = pe_t
                        for og in range(deq_og):
                            while (
                                tp_cnt[og] < len(segs[og])
                                and d_seg[(og, tp_cnt[og])] <= t_cur - 0.5
                                and drained + 0.9 <= idle
                            ):
                                emit_tp_seg(og)
                                drained += 0.9
                                t_cur += 0.9
                    if len(members) == 1 and min(bc_done) < 8 and drained < 2.0:
                        # xb-paced solo rounds idle ~6us -> HAM re-throttles;
                        # burn ~3us of dummy matmuls to keep the PE warm
                        emit_warm(14, BC)
                    round_ends.append(pe_t)
                    r += 1
            else:
                # original lockstep wavefront (whole-og dequant groups)
                s_of = list(range(OTILES))
                total_rounds = OTILES - 1 + NBC  # 18
                q_prefetch(4)
                emit_deq_og(0)
                for r in range(total_rounds):
                    if r + 1 < OTILES:
                        emit_deq_og(r + 1)
                    bc = r % NBC
                    units = [og for og in range(OTILES) if s_of[og] <= r < s_of[og] + NBC]
                    if not units:
                        continue
                    xb = emit_xblock(bc)
                    for og in units:
                        emit_mm(og, bc, xb)
    nc.finalize()
    return nc


_STATE = {}


def _get_compiled(opt=None):
    if "cb" in _STATE:
        return _STATE["cb"]
    import jax
    from jax.sharding import Mesh, PartitionSpec, NamedSharding
    from jax.experimental.shard_map import shard_map
    from concourse.bass2jax import (
        _bass_exec_p,
        install_neuronx_cc_hook,
        partition_id_tensor,
    )

    try:
        jax.config.update("jax_compilation_cache_dir", "/tmp/.anyprec_jaxcache")
        jax.config.update("jax_persistent_cache_min_compile_time_secs", 10)
        jax.config.update("jax_persistent_cache_min_entry_size_bytes", 0)
    except Exception:
        pass

    install_neuronx_cc_hook()
    nc = _build(opt)

    partition_name = nc.partition_id_tensor.name if nc.partition_id_tensor else None
    in_names, out_names, out_avals = [], [], []
    for alloc in nc.m.functions[0].allocations:
        if not isinstance(alloc, mybir.MemoryLocationSet):
            continue
        name = alloc.memorylocations[0].name
        if alloc.kind == "ExternalInput":
            if name != partition_name:
                in_names.append(name)
        elif alloc.kind == "ExternalOutput":
            out_names.append(name)
            out_avals.append(
                jax.core.ShapedArray(tuple(alloc.tensor_shape), mybir.dt.np(alloc.dtype))
            )
    all_in_names = in_names + out_names
    if partition_name is not None:
        all_in_names.append(partition_name)

    def _body(*args):
        operands = list(args)
        if partition_name is not None:
            operands.append(partition_id_tensor())
        return tuple(
            _bass_exec_p.bind(
                *operands,
                out_avals=tuple(out_avals),
                in_names=tuple(all_in_names),
                out_names=tuple(out_names),
                lowering_input_output_aliases=(),
                sim_require_finite=True,
                sim_require_nnan=True,
                nc=nc,
            )
        )

    devices = jax.devices()[:NCORES]
    mesh = Mesh(np.asarray(devices), ("core",))
    nin = len(in_names) + len(out_names)
    fn = jax.jit(
        shard_map(
            _body,
            mesh=mesh,
            in_specs=(PartitionSpec("core"),) * nin,
            out_specs=(PartitionSpec("core"),) * len(out_names),
            check_rep=False,
        ),
        keep_unused=True,
    )
    cb = {
        "fn": fn,
        "in_names": in_names,
        "out_names": out_names,
        "out_avals": out_avals,
        "sharding": NamedSharding(mesh, PartitionSpec("core")),
        "jax": jax,
    }
    _STATE["cb"] = cb
    return cb


def prepare_inputs(x, lut, qweight, x_layout="tiled"):
    x = np.asarray(x)
    lut = np.asarray(lut)
    qweight = np.asarray(qweight)
    xt = np.ascontiguousarray(x.astype(np.float16).T)  # [IN, BATCH]
    if x_layout == "tiled":
        # xt2[p, bc, i, b] = xt[i*128+p, bc*BC+b] -> contiguous xb loads
        xt = np.ascontiguousarray(
            xt.reshape(ITILES, 128, NBC, BC)
            .transpose(1, 2, 0, 3)
            .reshape(128, NBC * ITILES * BC)
        )
    qf_full = qweight.astype(np.float16)  # exact for 0..15
    lut_full = lut.astype(np.float32)

    xt_cat = np.concatenate([xt] * NCORES, axis=0)
    qf_cat = np.zeros((NCORES * OUT_PAD, IN), np.float16)
    lut_cat = np.zeros((NCORES * 128, OTILES * NK), np.float32)
    for c in range(NCORES):
        r0, r1 = c * OUT_SLICE, (c + 1) * OUT_SLICE
        qf_cat[c * OUT_PAD : c * OUT_PAD + OUT_SLICE] = qf_full[r0:r1]
        # partition-major lut pack: lut_cat[c*128+p, og*NK+k] = lut[og*128+p, k]
        lp = np.zeros((OUT_PAD, NK), np.float32)
        lp[:OUT_SLICE] = lut_full[r0:r1]
        lut_cat[c * 128 : (c + 1) * 128] = (
            lp.reshape(OTILES, 128, NK).transpose(1, 0, 2).reshape(128, OTILES * NK)
        )
    return {"xt": xt_cat, "qf": qf_cat, "lut": lut_cat}


def run_device(arrs, bench_reps=0, opt=None):
    cb = _get_compiled(opt)
    jax = cb["jax"]
    dev_args = [jax.device_put(arrs[n], cb["sharding"]) for n in cb["in_names"]] + [
        jax.device_put(
            np.zeros((NCORES * a.shape[0], *a.shape[1:]), a.dtype), cb["sharding"]
        )
        for a in cb["out_avals"]
    ]
    jax.block_until_ready(dev_args)
    outs = cb["fn"](*dev_args)
    jax.block_until_ready(outs)
    result = np.asarray(outs[0])  # [8*OUT_PAD, BATCH] fp16

    timing = None
    if bench_reps:
        import time

        def run_n(n):
            best = None
            for _ in range(2):
                t0 = time.perf_counter()
                o = None
                for _ in range(n):
                    o = cb["fn"](*dev_args)
                jax.block_until_ready(o)
                dt = time.perf_counter() - t0
                best = dt if best is None else min(best, dt)
            return best

        n1, n2 = 10, 10 + bench_reps
        t1, t2 = run_n(n1), run_n(n2)
        timing = (t2 - t1) / (n2 - n1)
    return result, timing


def kernel(x, lut, qweight, w_bits=4, _bench_reps=0, _opt=None):
    xl = {**OPT, **(_opt or {})}["x_layout"]
    arrs = prepare_inputs(x, lut, qweight, x_layout=xl)
    yt_cat, timing = run_device(arrs, bench_reps=_bench_reps, opt=_opt)
    yt = yt_cat.reshape(NCORES, OUT_PAD, BATCH)[:, :OUT_SLICE, :].reshape(OUT, BATCH)
    y = np.ascontiguousarray(yt.T)  # [BATCH, OUT] fp16
    if _bench_reps:
        kernel._last_timing = timing
    return y
